# revision 1
# baseline (speedup 1.0000x reference)
"""Drosophila optic lobe circuit simulation on 8 Trainium2 NeuronCores.

Edge/target-sharded across 8 devices; batch rides partitions.
- N padded 49000->49152 = 8 dev x 8 cores x 768 targets.
- Gather tables: partition p = 16k + 8h + b holds r=relu(v) of batch b,
  source-half h ([24576] fp32). ap_gather per 1600-slot chunk fetches
  r[src] for all 8 batches; wrong-half rows masked by wmask=0.
- currents = gathered * wmask (fp16 static weights streamed from DRAM).
- Scatter-add = carried inclusive cumsum (tensor_tensor_scan) + boundary
  extraction (small ap_gather per chunk) + first difference.
- Source halves folded by a static 0/1 matmul; v updated in extract-column
  layout; r re-sharded to id order (ap_gather), AllGathered across devices,
  tables rebuilt by broadcast DMAs.
"""

import numpy as np
import sys

sys.path.insert(0, "/opt/trn_rl_repo")

import concourse.bacc as bacc
import concourse.mybir as mybir
from concourse.tile import TileContext
from concourse.bass_utils import run_bass_kernel_spmd

NREAL = 49000
B = 8
DT = 0.1
NDEV = 8
N = 49152
NH = N // 2
NDEVT = N // NDEV          # 6144
NCORES = 8
MCORE = NDEVT // NCORES    # 768
NCHUNK = 16
CHUNK = 1600
LCORE = NCHUNK * CHUNK     # 25600
BCH = 96
ECOLS = NCHUNK * BCH       # 1280

_cache = {}


def _build(steps, with_bias, use_for_i=False, debug=False):
    nc = bacc.Bacc(None)
    f32, f16, i16 = mybir.dt.float32, mybir.dt.float16, mybir.dt.int16

    tbl0_in = nc.declare_dram_parameter("tbl0", [128, NH], f32, isOutput=False)
    idx1_in = nc.declare_dram_parameter("idx1", [128, LCORE // 16], i16, isOutput=False)
    wmask_in = nc.declare_dram_parameter("wmask", [128, LCORE], f16, isOutput=False)
    bidx_in = nc.declare_dram_parameter("bidx", [128, ECOLS // 16], i16, isOutput=False)
    idx3_in = nc.declare_dram_parameter("idx3", [128, MCORE // 16], i16, isOutput=False)
    v0_in = nc.declare_dram_parameter("v0", [128, ECOLS], f32, isOutput=False)
    mdt_in = nc.declare_dram_parameter("mdt", [128, ECOLS], f32, isOutput=False)
    fold_in = nc.declare_dram_parameter("fold", [128, 128], f32, isOutput=False)
    if with_bias:
        bm_in = nc.declare_dram_parameter("bm", [128, ECOLS], f32, isOutput=False)
    vout = nc.declare_dram_parameter("vout", [B, NDEVT], f32, isOutput=True)
    if debug:
        dbgE = nc.declare_dram_parameter("dbgE", [128, ECOLS], f32, isOutput=True)
        dbgG = nc.declare_dram_parameter("dbgG", [128, CHUNK], f32, isOutput=True)
        dbgS = nc.declare_dram_parameter("dbgS", [128, CHUNK], f32, isOutput=True)

    with TileContext(nc) as tc:
        with (
            tc.tile_pool(name="big", bufs=1) as big,
            tc.tile_pool(name="gbuf", bufs=2) as gbuf,
            tc.tile_pool(name="wbuf", bufs=2) as wbuf,
            tc.tile_pool(name="cs", bufs=2) as csp,
            tc.tile_pool(name="small", bufs=1) as small,
            tc.tile_pool(name="psum", bufs=1, space="PSUM") as psum,
            tc.tile_pool(name="dram", bufs=1, space="DRAM") as dram,
            tc.tile_pool(name="agpool", bufs=max(steps, 1), space="DRAM") as agp,
        ):
            tbl = big.tile([128, NH], f32, tag="tbl")
            idx1 = small.tile([128, LCORE // 16], i16, tag="idx1")
            bidx = small.tile([128, ECOLS // 16], i16, tag="bidx")
            idx3 = small.tile([128, MCORE // 16], i16, tag="idx3")
            v = small.tile([128, ECOLS], f32, tag="v")
            mdt = small.tile([128, ECOLS], f32, tag="mdt")
            fold = small.tile([128, 128], f32, tag="fold")
            bm = small.tile([128, ECOLS], f32, tag="bm") if with_bias else None
            E = small.tile([128, ECOLS], f32, tag="E")
            syn = small.tile([128, ECOLS], f32, tag="syn")
            carry = small.tile([128, 1], f32, tag="carry")
            rslab = small.tile([128, MCORE], f32, tag="rslab")
            vslab = small.tile([128, MCORE], f32, tag="vslab")

            slab_d = dram.tile([B, NDEVT], f32)

            nc.sync.dma_start(out=tbl[:], in_=tbl0_in[:])
            nc.sync.dma_start(out=idx1[:], in_=idx1_in[:])
            nc.sync.dma_start(out=bidx[:], in_=bidx_in[:])
            nc.sync.dma_start(out=idx3[:], in_=idx3_in[:])
            nc.sync.dma_start(out=v[:], in_=v0_in[:])
            nc.sync.dma_start(out=mdt[:], in_=mdt_in[:])
            nc.sync.dma_start(out=fold[:], in_=fold_in[:])
            if with_bias:
                nc.sync.dma_start(out=bm[:], in_=bm_in[:])
            nc.vector.memset(carry[:], 0.0)

            def step_body(_iv=None):
                s_tiles = {}
                for c in range(NCHUNK):
                    g = gbuf.tile([128, CHUNK], f32, tag="g")
                    w = wbuf.tile([128, CHUNK], f16, tag="w")
                    cur = csp.tile([128, CHUNK], f32, tag="cur")
                    s = csp.tile([128, CHUNK], f32, tag="s")
                    s_tiles[c] = s
                    nc.sync.dma_start(out=w[:], in_=wmask_in[:, c * CHUNK:(c + 1) * CHUNK])
                    nc.gpsimd.ap_gather(
                        out_ap=g[:], in_ap=tbl[:],
                        idxs_ap=idx1[:, c * (CHUNK // 16):(c + 1) * (CHUNK // 16)],
                        channels=128, num_elems=NH, d=1, num_idxs=CHUNK,
                    )
                    # extract previous chunk's boundaries (after this gather so
                    # POOL doesn't stall on the DVE scan)
                    if c >= 1:
                        sp = s_tiles.pop(c - 1)
                        nc.gpsimd.ap_gather(
                            out_ap=E[:, (c - 1) * BCH:c * BCH], in_ap=sp[:],
                            idxs_ap=bidx[:, (c - 1) * (BCH // 16):c * (BCH // 16)],
                            channels=128, num_elems=CHUNK, d=1, num_idxs=BCH,
                        )
                    if debug and c == 0:
                        nc.sync.dma_start(out=dbgG[:], in_=g[:])
                    nc.vector.tensor_tensor(out=cur[:], in0=g[:], in1=w[:],
                                            op=mybir.AluOpType.mult)
                    init = 0.0 if c == 0 else carry[:, 0:1]
                    nc.vector.tensor_tensor_scan(
                        out=s[:], data0=cur[:], data1=cur[:], initial=init,
                        op0=mybir.AluOpType.add, op1=mybir.AluOpType.bypass,
                    )
                    if debug and c == 1:
                        nc.sync.dma_start(out=dbgS[:], in_=s[:])
                    if c < NCHUNK - 1:
                        nc.vector.tensor_copy(out=carry[:], in_=s[:, CHUNK - 1:CHUNK])
                c = NCHUNK
                sp = s_tiles.pop(c - 1)
                nc.gpsimd.ap_gather(
                    out_ap=E[:, (c - 1) * BCH:c * BCH], in_ap=sp[:],
                    idxs_ap=bidx[:, (c - 1) * (BCH // 16):c * (BCH // 16)],
                    channels=128, num_elems=CHUNK, d=1, num_idxs=BCH,
                )
                # fold halves: Ef = fold.T @ E
                if debug:
                    nc.sync.dma_start(out=dbgE[:], in_=E[:])
                ef = psum.tile([128, ECOLS], f32, tag="ef")
                for j in range(0, ECOLS, 512):
                    jw = min(512, ECOLS - j)
                    nc.tensor.matmul(out=ef[:, j:j + jw], lhsT=fold[:],
                                     rhs=E[:, j:j + jw], start=True, stop=True)
                # segment sums by first difference (via SBUF copy of ef)
                nc.vector.tensor_copy(out=syn[:], in_=ef[:])
                nc.vector.tensor_copy(out=E[:, 0:1], in_=syn[:, 0:1])
                nc.vector.tensor_tensor(out=E[:, 1:ECOLS], in0=syn[:, 1:ECOLS],
                                        in1=syn[:, 0:ECOLS - 1],
                                        op=mybir.AluOpType.subtract)
                # v += mdt * (E - v) (+ bm)
                nc.vector.tensor_tensor(out=E[:], in0=E[:], in1=v[:],
                                        op=mybir.AluOpType.subtract)
                nc.vector.tensor_tensor(out=E[:], in0=E[:], in1=mdt[:],
                                        op=mybir.AluOpType.mult)
                nc.vector.tensor_tensor(out=v[:], in0=v[:], in1=E[:],
                                        op=mybir.AluOpType.add)
                if with_bias:
                    nc.vector.tensor_tensor(out=v[:], in0=v[:], in1=bm[:],
                                            op=mybir.AluOpType.add)
                # r = relu(v) -> id-order slab -> DRAM -> AllGather -> tables
                nc.scalar.activation(syn[:], v[:], mybir.ActivationFunctionType.Relu)
                nc.gpsimd.ap_gather(out_ap=rslab[:], in_ap=syn[:], idxs_ap=idx3[:],
                                    channels=128, num_elems=ECOLS, d=1, num_idxs=MCORE)
                for k in range(NCORES):
                    nc.sync.dma_start(out=slab_d[:, k * MCORE:(k + 1) * MCORE],
                                      in_=rslab[16 * k:16 * k + 8, :])
                ag_d = agp.tile([NDEV * B, NDEVT], f32, addr_space="Shared", tag="ag")
                nc.gpsimd.collective_compute(
                    "AllGather", mybir.AluOpType.bypass,
                    replica_groups=[list(range(NDEV))],
                    ins=[slab_d[:]], outs=[ag_d[:]],
                )
                agv = ag_d[:].rearrange("(d b) n -> d b n", d=NDEV)
                for h in range(2):
                    for b in range(B):
                        nc.sync.dma_start(
                            out=tbl[:].rearrange("(k r) n -> k r n", k=8)[:, 8 * h + b, :],
                            in_=agv[4 * h:4 * h + 4, b, :][None]
                                .to_broadcast([8, 4, NDEVT]),
                        )

            if steps == 1:
                step_body()
            elif steps > 1:
                if use_for_i:
                    with tc.For_i(0, steps, 1) as iv:
                        step_body(iv)
                else:
                    for _ in range(steps):
                        step_body()

            nc.gpsimd.ap_gather(out_ap=vslab[:], in_ap=v[:], idxs_ap=idx3[:],
                                channels=128, num_elems=ECOLS, d=1, num_idxs=MCORE)
            for k in range(NCORES):
                nc.sync.dma_start(out=vout[:, k * MCORE:(k + 1) * MCORE],
                                  in_=vslab[16 * k:16 * k + 8, :])
    nc.finalize()
    return nc


def _wrap16(a):
    out = np.zeros((128, a.shape[1] // 16), a.dtype)
    for k in range(8):
        for p in range(16):
            out[16 * k + p] = a[k, p::16]
    return out


def _prep(tm1_input, v_init, weights, bias, tau_params, scale_excitatory,
          scale_inhibitory, source_indices, target_indices, type_ids,
          tm1_indices, steps):
    one = np.float32(1.0)
    weights = np.asarray(weights, np.float32)
    es = np.where(weights > 0, np.float32(scale_excitatory),
                  np.where(weights < 0, np.float32(scale_inhibitory), one))
    sw = (weights * es).astype(np.float32)

    type_ids = np.asarray(type_ids)
    tau = np.asarray(tau_params, np.float32)[type_ids]
    taup = np.concatenate([tau, np.full(N - NREAL, 1.0, np.float32)])
    is_tm1 = np.zeros(N, bool)
    tm1_indices = np.asarray(tm1_indices)
    is_tm1[tm1_indices] = True
    biasp = np.zeros(N, np.float32)
    biasp[:NREAL] = np.asarray(bias, np.float32)

    vc = np.zeros((B, N), np.float32)
    vc[:, :NREAL] = np.asarray(v_init, np.float32)
    vc[:, tm1_indices] = np.asarray(tm1_input, np.float32)

    order = np.argsort(target_indices, kind="stable")
    tsrc = np.asarray(source_indices)[order].astype(np.int64)
    tw = sw[order]
    ttgt = np.asarray(target_indices)[order].astype(np.int64)
    t_starts = np.searchsorted(ttgt, np.arange(N + 1, dtype=np.int64), side="left")

    r0 = np.maximum(vc, 0.0)
    tbl0 = np.zeros((128, NH), np.float32)
    for k in range(NCORES):
        for h in range(2):
            for b in range(B):
                tbl0[16 * k + 8 * h + b] = r0[b, h * NH:(h + 1) * NH]
    F = np.zeros((128, 128), np.float32)
    for p in range(128):
        for m in range(128):
            if p // 16 == m // 16 and p % 8 == m % 8:
                F[p, m] = 1.0

    in_maps = []
    meta = []
    for d in range(NDEV):
        idx1 = np.zeros((8, LCORE), np.int16)
        wm = np.zeros((128, LCORE), np.float16)
        bpos = np.zeros((8, ECOLS), np.int16)
        col_of_t = np.zeros((8, MCORE), np.int64)
        for k in range(NCORES):
            t0 = d * NDEVT + k * MCORE
            e0, e1 = t_starts[t0], t_starts[t0 + MCORE]
            srcs = tsrc[e0:e1]
            ws = tw[e0:e1]
            counts = t_starts[t0 + 1:t0 + MCORE + 1] - t_starts[t0:t0 + MCORE]
            pos = np.cumsum(counts)              # extract position per target
            nslots = 1 + len(srcs)               # sentinel at slot 0
            assert nslots <= LCORE, f"core slots {nslots} > {LCORE}"
            idx1[k, 1:nslots] = (srcs % NH).astype(np.int16)
            half = np.zeros(LCORE, np.int64)
            wrow = np.zeros(LCORE, np.float32)
            half[1:nslots] = srcs // NH
            wrow[1:nslots] = ws
            for h in range(2):
                wh = np.where(half == h, wrow, 0.0).astype(np.float16)
                for b in range(B):
                    wm[16 * k + 8 * h + b] = wh
            # boundary extraction, chunked (real targets only; virtual
            # padding targets share the final pad column: syn there is
            # garbage but mdt=0 and v0=0 keep their state at 0)
            ids_k = d * NDEVT + k * MCORE + np.arange(MCORE)
            cchunk = pos // CHUNK
            clocal = pos % CHUNK
            ci = 0
            for c in range(NCHUNK):
                nhere = 0
                while ci < MCORE and cchunk[ci] == c:
                    if ids_k[ci] >= NREAL:
                        col_of_t[k, ci] = ECOLS - 1
                        ci += 1
                        continue
                    assert nhere < BCH - 1, f"chunk {c} boundary overflow"
                    bpos[k, c * BCH + nhere] = clocal[ci]
                    col_of_t[k, ci] = c * BCH + nhere
                    nhere += 1
                    ci += 1
                padv = bpos[k, c * BCH + nhere - 1] if nhere else 0
                bpos[k, c * BCH + nhere:(c + 1) * BCH] = padv
            assert ci == MCORE
        gids = (d * NDEVT + np.arange(NDEVT)).reshape(NCORES, MCORE)
        v0 = np.zeros((128, ECOLS), np.float32)
        mdt = np.zeros((128, ECOLS), np.float32)
        bmt = np.zeros((128, ECOLS), np.float32)
        for k in range(NCORES):
            cols = col_of_t[k]
            ids = gids[k]
            upd = (~is_tm1[ids]) & (ids < NREAL)
            mvals = np.where(upd, DT / taup[ids], 0.0).astype(np.float32)
            bvals = (mvals * biasp[ids]).astype(np.float32)
            for h in range(2):
                for b in range(B):
                    p = 16 * k + 8 * h + b
                    v0[p, cols] = vc[b, ids]
                    mdt[p, cols] = mvals
                    bmt[p, cols] = bvals
        in_maps.append({
            "tbl0": tbl0, "idx1": _wrap16(idx1), "wmask": wm,
            "bidx": _wrap16(bpos), "idx3": _wrap16(col_of_t.astype(np.int16)),
            "v0": v0, "mdt": mdt, "fold": F, "bm": bmt,
        })
        meta.append(col_of_t)
    return in_maps, meta


def kernel(**inputs):
    steps = int(inputs["steps"])
    bias = np.asarray(inputs["bias"])
    with_bias = bool(np.any(bias != 0))
    in_maps, _meta = _prep(**inputs)
    if not with_bias:
        for m in in_maps:
            m.pop("bm")
    key = (steps, with_bias)
    if key not in _cache:
        _cache[key] = _build(steps, with_bias)
    nc = _cache[key]
    res = run_bass_kernel_spmd(nc, in_maps, list(range(NDEV)))
    out = np.zeros((B, NREAL), np.float32)
    for d in range(NDEV):
        sl = res.results[d]["vout"]
        lo = d * NDEVT
        hi = min(lo + NDEVT, NREAL)
        out[:, lo:hi] = sl[:, :hi - lo]
    return out



# revision 3
# speedup vs baseline: 2.4586x; 2.4586x over previous
"""Drosophila optic lobe circuit simulation on 8 Trainium2 NeuronCores.

Edge/target-sharded across 8 devices; batch rides partitions.
- N padded 49000->49152 = 8 dev x 8 cores x 768 targets.
- Gather tables: partition p = 16k + 8h + b holds r=relu(v) of batch b,
  source-half h ([24576] fp32). ap_gather per 1600-slot chunk fetches
  r[src] for all 8 batches; wrong-half rows masked by wmask=0.
- currents = gathered * wmask (fp16 static weights streamed from DRAM).
- Scatter-add = carried inclusive cumsum (tensor_tensor_scan) + boundary
  extraction (small ap_gather per chunk) + first difference.
- Source halves folded by a static 0/1 matmul; v updated in extract-column
  layout; r re-sharded to id order (ap_gather), AllGathered across devices,
  tables rebuilt by broadcast DMAs.

Host->device transfer over the axon tunnel is the wall-clock bottleneck
and scales with the LARGEST single parameter, not total bytes (arrays
stream in parallel). So inputs are shipped deduplicated (weights are
batch-invariant: 16 unique rows, not 128; v0 is half-invariant; mdt is
per-core) and split into ~200KB chunks. The r-table (12.6MB, formerly an
input) is built on device by running the publish path once before the
loop. Weights are expanded once into device DRAM (wmx) and streamed
per-chunk from there each step, keeping the steady-state instruction
stream unchanged.
"""

import numpy as np
import sys

sys.path.insert(0, "/opt/trn_rl_repo")

import concourse.bacc as bacc
import concourse.mybir as mybir
from concourse.tile import TileContext
from concourse.bass_utils import run_bass_kernel_spmd

NREAL = 49000
B = 8
DT = 0.1
NDEV = 8
N = 49152
NH = N // 2
NDEVT = N // NDEV          # 6144
NCORES = 8
MCORE = NDEVT // NCORES    # 768
NCHUNK = 16
CHUNK = 1600
LCORE = NCHUNK * CHUNK     # 25600
BCH = 96
ECOLS = NCHUNK * BCH       # 1280

_cache = {}


def _build(steps, with_bias):
    nc = bacc.Bacc(None)
    f32, f16, i16 = mybir.dt.float32, mybir.dt.float16, mybir.dt.int16

    # compact, split inputs (transfer wall ~ largest single param)
    wm_in = [nc.declare_dram_parameter(f"wm{j}", [16, CHUNK // 2], f16,
                                       isOutput=False) for j in range(2 * NCHUNK)]
    ix_in = [nc.declare_dram_parameter(f"ix{c}", [128, CHUNK // 16], i16,
                                       isOutput=False) for c in range(NCHUNK)]
    v0_in = [nc.declare_dram_parameter(f"v0{j}", [8, ECOLS // 2], f32,
                                       isOutput=False) for j in range(16)]
    fold_in = [nc.declare_dram_parameter(f"fold{j}", [32, 128], f32,
                                         isOutput=False) for j in range(4)]
    mdt_in = [nc.declare_dram_parameter(f"mdt{j}", [8, ECOLS // 2], f32,
                                        isOutput=False) for j in range(2)]
    if with_bias:
        bm_in = [nc.declare_dram_parameter(f"bm{j}", [8, ECOLS // 2], f32,
                                           isOutput=False) for j in range(2)]
    bidx_in = nc.declare_dram_parameter("bidx", [128, ECOLS // 16], i16, isOutput=False)
    idx3_in = nc.declare_dram_parameter("idx3", [128, MCORE // 16], i16, isOutput=False)
    vout = [nc.declare_dram_parameter(f"vout{k}", [B, MCORE], f32, isOutput=True)
            for k in range(NCORES)]

    with TileContext(nc) as tc:
        with (
            tc.tile_pool(name="big", bufs=1) as big,
            tc.tile_pool(name="gbuf", bufs=2) as gbuf,
            tc.tile_pool(name="wbuf", bufs=2) as wbuf,
            tc.tile_pool(name="cs", bufs=2) as csp,
            tc.tile_pool(name="small", bufs=1) as small,
            tc.tile_pool(name="psum", bufs=1, space="PSUM") as psum,
            tc.tile_pool(name="dram", bufs=1, space="DRAM") as dram,
            tc.tile_pool(name="agpool", bufs=max(steps, 1), space="DRAM") as agp,
        ):
            tbl = big.tile([128, NH], f32, tag="tbl")
            idx1 = small.tile([128, LCORE // 16], i16, tag="idx1")
            bidx = small.tile([128, ECOLS // 16], i16, tag="bidx")
            idx3 = small.tile([128, MCORE // 16], i16, tag="idx3")
            v = small.tile([128, ECOLS], f32, tag="v")
            mdt = small.tile([128, ECOLS], f32, tag="mdt")
            fold = small.tile([128, 128], f32, tag="fold")
            bm = small.tile([128, ECOLS], f32, tag="bm") if with_bias else None
            E = small.tile([128, ECOLS], f32, tag="E")
            syn = small.tile([128, ECOLS], f32, tag="syn")
            carry = small.tile([128, 1], f32, tag="carry")
            rslab = small.tile([128, MCORE], f32, tag="rslab")
            vslab = small.tile([128, MCORE], f32, tag="vslab")

            slab_d = dram.tile([B, NDEVT], f32)
            wmx = dram.tile([128, LCORE], f16)

            # ---- input loads / on-device expansion (one-time) ----
            for c in range(NCHUNK):
                nc.sync.dma_start(out=idx1[:, c * 100:(c + 1) * 100], in_=ix_in[c][:])
            nc.sync.dma_start(out=bidx[:], in_=bidx_in[:])
            nc.sync.dma_start(out=idx3[:], in_=idx3_in[:])
            for j in range(4):
                nc.sync.dma_start(out=fold[32 * j:32 * (j + 1), :], in_=fold_in[j][:])
            HC = ECOLS // 2
            mdtv = mdt[:].rearrange("(k r) n -> k r n", k=8)
            for r in range(16):
                nc.sync.dma_start(out=mdtv[:, r, 0:HC], in_=mdt_in[0][:])
                nc.sync.dma_start(out=mdtv[:, r, HC:ECOLS], in_=mdt_in[1][:])
            if with_bias:
                bmv = bm[:].rearrange("(k r) n -> k r n", k=8)
                for r in range(16):
                    nc.sync.dma_start(out=bmv[:, r, 0:HC], in_=bm_in[0][:])
                    nc.sync.dma_start(out=bmv[:, r, HC:ECOLS], in_=bm_in[1][:])
            vv = v[:].rearrange("(k h b) n -> k h b n", k=8, h=2)
            for h in range(2):
                for b in range(B):
                    nc.sync.dma_start(out=vv[:, h, b, 0:HC], in_=v0_in[2 * b][:])
                    nc.sync.dma_start(out=vv[:, h, b, HC:ECOLS], in_=v0_in[2 * b + 1][:])
            # expand weights (batch-invariant rows) into DRAM wmx once
            for c in range(NCHUNK):
                w = wbuf.tile([128, CHUNK], f16, tag="w")
                wv = w[:].rearrange("(g b) n -> g b n", g=16)
                for b in range(B):
                    nc.sync.dma_start(out=wv[:, b, 0:CHUNK // 2], in_=wm_in[2 * c][:])
                    nc.sync.dma_start(out=wv[:, b, CHUNK // 2:CHUNK],
                                      in_=wm_in[2 * c + 1][:])
                nc.sync.dma_start(out=wmx[:, c * CHUNK:(c + 1) * CHUNK], in_=w[:])
            nc.vector.memset(carry[:], 0.0)

            def publish():
                # r = relu(v) -> id-order slab -> DRAM -> AllGather -> tables
                nc.scalar.activation(syn[:], v[:], mybir.ActivationFunctionType.Relu)
                nc.gpsimd.ap_gather(out_ap=rslab[:], in_ap=syn[:], idxs_ap=idx3[:],
                                    channels=128, num_elems=ECOLS, d=1, num_idxs=MCORE)
                for k in range(NCORES):
                    nc.sync.dma_start(out=slab_d[:, k * MCORE:(k + 1) * MCORE],
                                      in_=rslab[16 * k:16 * k + 8, :])
                ag_d = agp.tile([NDEV * B, NDEVT], f32, addr_space="Shared", tag="ag")
                nc.gpsimd.collective_compute(
                    "AllGather", mybir.AluOpType.bypass,
                    replica_groups=[list(range(NDEV))],
                    ins=[slab_d[:]], outs=[ag_d[:]],
                )
                agv = ag_d[:].rearrange("(d b) n -> d b n", d=NDEV)
                for h in range(2):
                    for b in range(B):
                        nc.sync.dma_start(
                            out=tbl[:].rearrange("(k r) n -> k r n", k=8)[:, 8 * h + b, :],
                            in_=agv[4 * h:4 * h + 4, b, :][None]
                                .to_broadcast([8, 4, NDEVT]),
                        )

            def step_body(last):
                s_tiles = {}
                for c in range(NCHUNK):
                    g = gbuf.tile([128, CHUNK], f32, tag="g")
                    w = wbuf.tile([128, CHUNK], f16, tag="w")
                    cur = csp.tile([128, CHUNK], f32, tag="cur")
                    s = csp.tile([128, CHUNK], f32, tag="s")
                    s_tiles[c] = s
                    nc.sync.dma_start(out=w[:], in_=wmx[:, c * CHUNK:(c + 1) * CHUNK])
                    nc.gpsimd.ap_gather(
                        out_ap=g[:], in_ap=tbl[:],
                        idxs_ap=idx1[:, c * (CHUNK // 16):(c + 1) * (CHUNK // 16)],
                        channels=128, num_elems=NH, d=1, num_idxs=CHUNK,
                    )
                    # extract previous chunk's boundaries (after this gather so
                    # POOL doesn't stall on the DVE scan)
                    if c >= 1:
                        sp = s_tiles.pop(c - 1)
                        nc.gpsimd.ap_gather(
                            out_ap=E[:, (c - 1) * BCH:c * BCH], in_ap=sp[:],
                            idxs_ap=bidx[:, (c - 1) * (BCH // 16):c * (BCH // 16)],
                            channels=128, num_elems=CHUNK, d=1, num_idxs=BCH,
                        )
                    nc.vector.tensor_tensor(out=cur[:], in0=g[:], in1=w[:],
                                            op=mybir.AluOpType.mult)
                    init = 0.0 if c == 0 else carry[:, 0:1]
                    nc.vector.tensor_tensor_scan(
                        out=s[:], data0=cur[:], data1=cur[:], initial=init,
                        op0=mybir.AluOpType.add, op1=mybir.AluOpType.bypass,
                    )
                    if c < NCHUNK - 1:
                        nc.vector.tensor_copy(out=carry[:], in_=s[:, CHUNK - 1:CHUNK])
                c = NCHUNK
                sp = s_tiles.pop(c - 1)
                nc.gpsimd.ap_gather(
                    out_ap=E[:, (c - 1) * BCH:c * BCH], in_ap=sp[:],
                    idxs_ap=bidx[:, (c - 1) * (BCH // 16):c * (BCH // 16)],
                    channels=128, num_elems=CHUNK, d=1, num_idxs=BCH,
                )
                # fold halves: Ef = fold.T @ E
                ef = psum.tile([128, ECOLS], f32, tag="ef")
                for j in range(0, ECOLS, 512):
                    jw = min(512, ECOLS - j)
                    nc.tensor.matmul(out=ef[:, j:j + jw], lhsT=fold[:],
                                     rhs=E[:, j:j + jw], start=True, stop=True)
                # segment sums by first difference (via SBUF copy of ef)
                nc.vector.tensor_copy(out=syn[:], in_=ef[:])
                nc.vector.tensor_copy(out=E[:, 0:1], in_=syn[:, 0:1])
                nc.vector.tensor_tensor(out=E[:, 1:ECOLS], in0=syn[:, 1:ECOLS],
                                        in1=syn[:, 0:ECOLS - 1],
                                        op=mybir.AluOpType.subtract)
                # v += mdt * (E - v) (+ bm)
                nc.vector.tensor_tensor(out=E[:], in0=E[:], in1=v[:],
                                        op=mybir.AluOpType.subtract)
                nc.vector.tensor_tensor(out=E[:], in0=E[:], in1=mdt[:],
                                        op=mybir.AluOpType.mult)
                nc.vector.tensor_tensor(out=v[:], in0=v[:], in1=E[:],
                                        op=mybir.AluOpType.add)
                if with_bias:
                    nc.vector.tensor_tensor(out=v[:], in0=v[:], in1=bm[:],
                                            op=mybir.AluOpType.add)
                if not last:
                    publish()

            if steps >= 1:
                publish()
                for s in range(steps):
                    step_body(last=(s == steps - 1))

            nc.gpsimd.ap_gather(out_ap=vslab[:], in_ap=v[:], idxs_ap=idx3[:],
                                channels=128, num_elems=ECOLS, d=1, num_idxs=MCORE)
            for k in range(NCORES):
                nc.sync.dma_start(out=vout[k][:], in_=vslab[16 * k:16 * k + 8, :])
    nc.finalize()
    return nc


def _wrap16(a):
    out = np.zeros((128, a.shape[1] // 16), a.dtype)
    for k in range(8):
        for p in range(16):
            out[16 * k + p] = a[k, p::16]
    return out


def _prep(tm1_input, v_init, weights, bias, tau_params, scale_excitatory,
          scale_inhibitory, source_indices, target_indices, type_ids,
          tm1_indices, steps):
    one = np.float32(1.0)
    weights = np.asarray(weights, np.float32)
    es = np.where(weights > 0, np.float32(scale_excitatory),
                  np.where(weights < 0, np.float32(scale_inhibitory), one))
    sw = (weights * es).astype(np.float32)

    type_ids = np.asarray(type_ids)
    tau = np.asarray(tau_params, np.float32)[type_ids]
    taup = np.concatenate([tau, np.full(N - NREAL, 1.0, np.float32)])
    is_tm1 = np.zeros(N, bool)
    tm1_indices = np.asarray(tm1_indices)
    is_tm1[tm1_indices] = True
    biasp = np.zeros(N, np.float32)
    biasp[:NREAL] = np.asarray(bias, np.float32)

    vc = np.zeros((B, N), np.float32)
    vc[:, :NREAL] = np.asarray(v_init, np.float32)
    vc[:, tm1_indices] = np.asarray(tm1_input, np.float32)

    order = np.argsort(target_indices, kind="stable")
    tsrc = np.asarray(source_indices)[order].astype(np.int64)
    tw = sw[order]
    ttgt = np.asarray(target_indices)[order].astype(np.int64)
    t_starts = np.searchsorted(ttgt, np.arange(N + 1, dtype=np.int64), side="left")

    F = np.zeros((128, 128), np.float32)
    for p in range(128):
        for m in range(128):
            if p // 16 == m // 16 and p % 8 == m % 8:
                F[p, m] = 1.0

    in_maps = []
    meta = []
    HC = ECOLS // 2
    for d in range(NDEV):
        idx1 = np.zeros((8, LCORE), np.int16)
        wmc = np.zeros((16, LCORE), np.float16)   # row g = 2k + h
        bpos = np.zeros((8, ECOLS), np.int16)
        col_of_t = np.zeros((8, MCORE), np.int64)
        for k in range(NCORES):
            t0 = d * NDEVT + k * MCORE
            e0, e1 = t_starts[t0], t_starts[t0 + MCORE]
            srcs = tsrc[e0:e1]
            ws = tw[e0:e1]
            counts = t_starts[t0 + 1:t0 + MCORE + 1] - t_starts[t0:t0 + MCORE]
            pos = np.cumsum(counts)              # extract position per target
            nslots = 1 + len(srcs)               # sentinel at slot 0
            assert nslots <= LCORE, f"core slots {nslots} > {LCORE}"
            idx1[k, 1:nslots] = (srcs % NH).astype(np.int16)
            half = np.zeros(LCORE, np.int64)
            wrow = np.zeros(LCORE, np.float32)
            half[1:nslots] = srcs // NH
            wrow[1:nslots] = ws
            for h in range(2):
                wmc[2 * k + h] = np.where(half == h, wrow, 0.0).astype(np.float16)
            # boundary extraction, chunked (real targets only; virtual
            # padding targets share the final pad column: syn there is
            # garbage but mdt=0 and v0=0 keep their state at 0)
            ids_k = d * NDEVT + k * MCORE + np.arange(MCORE)
            cchunk = pos // CHUNK
            clocal = pos % CHUNK
            ci = 0
            for c in range(NCHUNK):
                nhere = 0
                while ci < MCORE and cchunk[ci] == c:
                    if ids_k[ci] >= NREAL:
                        col_of_t[k, ci] = ECOLS - 1
                        ci += 1
                        continue
                    assert nhere < BCH - 1, f"chunk {c} boundary overflow"
                    bpos[k, c * BCH + nhere] = clocal[ci]
                    col_of_t[k, ci] = c * BCH + nhere
                    nhere += 1
                    ci += 1
                padv = bpos[k, c * BCH + nhere - 1] if nhere else 0
                bpos[k, c * BCH + nhere:(c + 1) * BCH] = padv
            assert ci == MCORE
        gids = (d * NDEVT + np.arange(NDEVT)).reshape(NCORES, MCORE)
        v0 = np.zeros((8 * B, ECOLS), np.float32)   # row 8k + b
        mdt = np.zeros((8, ECOLS), np.float32)      # row k
        bmt = np.zeros((8, ECOLS), np.float32)
        for k in range(NCORES):
            cols = col_of_t[k]
            ids = gids[k]
            upd = (~is_tm1[ids]) & (ids < NREAL)
            mvals = np.where(upd, DT / taup[ids], 0.0).astype(np.float32)
            mdt[k, cols] = mvals
            bmt[k, cols] = (mvals * biasp[ids]).astype(np.float32)
            for b in range(B):
                v0[8 * k + b, cols] = vc[b, ids]
        m = {
            "bidx": _wrap16(bpos), "idx3": _wrap16(col_of_t.astype(np.int16)),
        }
        ix_w = _wrap16(idx1)
        for c in range(NCHUNK):
            m[f"ix{c}"] = np.ascontiguousarray(ix_w[:, c * 100:(c + 1) * 100])
        for j in range(2 * NCHUNK):
            m[f"wm{j}"] = np.ascontiguousarray(
                wmc[:, j * (CHUNK // 2):(j + 1) * (CHUNK // 2)])
        for b in range(B):
            vb = v0[np.arange(NCORES) * 8 + b]
            m[f"v0{2 * b}"] = np.ascontiguousarray(vb[:, 0:HC])
            m[f"v0{2 * b + 1}"] = np.ascontiguousarray(vb[:, HC:ECOLS])
        for j in range(4):
            m[f"fold{j}"] = np.ascontiguousarray(F[32 * j:32 * (j + 1), :])
        m["mdt0"] = np.ascontiguousarray(mdt[:, 0:HC])
        m["mdt1"] = np.ascontiguousarray(mdt[:, HC:ECOLS])
        m["bm0"] = np.ascontiguousarray(bmt[:, 0:HC])
        m["bm1"] = np.ascontiguousarray(bmt[:, HC:ECOLS])
        in_maps.append(m)
        meta.append(col_of_t)
    return in_maps, meta


def kernel(**inputs):
    steps = int(inputs["steps"])
    bias = np.asarray(inputs["bias"])
    with_bias = bool(np.any(bias != 0))
    in_maps, _meta = _prep(**inputs)
    if not with_bias:
        for m in in_maps:
            m.pop("bm0")
            m.pop("bm1")
    key = (steps, with_bias)
    if key not in _cache:
        _cache[key] = _build(steps, with_bias)
    nc = _cache[key]
    res = run_bass_kernel_spmd(nc, in_maps, list(range(NDEV)))
    out = np.zeros((B, NREAL), np.float32)
    for d in range(NDEV):
        for k in range(NCORES):
            lo = d * NDEVT + k * MCORE
            hi = min(lo + MCORE, NREAL)
            if hi > lo:
                out[:, lo:hi] = res.results[d][f"vout{k}"][:, :hi - lo]
    return out


# revision 13
# speedup vs baseline: 5.6738x; 2.3078x over previous
"""Drosophila optic lobe circuit simulation on 8 Trainium2 NeuronCores.

Edge/target-sharded across 8 devices; batch rides partitions.
- N padded 49000->49152 = 8 dev x 8 cores x 768 targets.
- Gather tables: partition p = 16k + 8h + b holds r=relu(v) of batch b,
  source-half h ([24576] fp32). ap_gather per 1600-slot chunk fetches
  r[src] for all 8 batches; wrong-half rows masked by wmask=0.
- currents = gathered * wmask (fp16 static weights streamed from DRAM).
- Scatter-add = carried inclusive cumsum (tensor_tensor_scan) + boundary
  extraction (small ap_gather per chunk) + first difference.
- Source halves folded by a static 0/1 matmul; v updated in extract-column
  layout; r re-sharded to id order (ap_gather), AllGathered across devices,
  tables rebuilt by broadcast DMAs.

Host->device transfer over the axon tunnel is the wall-clock bottleneck
and scales with the LARGEST single parameter, not total bytes (arrays
stream in parallel). So inputs are shipped deduplicated (weights are
batch-invariant: 16 unique rows, not 128; v0 is half-invariant; mdt is
per-core) and split into ~200KB chunks. The r-table (12.6MB, formerly an
input) is built on device by running the publish path once before the
loop. Weights are expanded once into device DRAM (wmx) and streamed
per-chunk from there each step, keeping the steady-state instruction
stream unchanged.
"""

import numpy as np
import sys

sys.path.insert(0, "/opt/trn_rl_repo")

import jax

import concourse.bacc as bacc
import concourse.mybir as mybir
from concourse.tile import TileContext
from concourse.bass_utils import run_bass_kernel_spmd

# Cache XLA executables on disk so repeat dispatches skip the re-compile
# path (the jit closure inside run_bass_via_pjrt is fresh per call, so
# jax's in-memory caches never hit).
jax.config.update("jax_compilation_cache_dir", "/tmp/jax_cache_bass")
jax.config.update("jax_persistent_cache_min_compile_time_secs", 0.0)
jax.config.update("jax_persistent_cache_min_entry_size_bytes", 0)

NREAL = 49000
B = 8
DT = 0.1
NDEV = 8
N = 49152
NH = N // 2
NDEVT = N // NDEV          # 6144
NCORES = 8
MCORE = NDEVT // NCORES    # 768
NCHUNK = 16
CHUNK = 1600
LCORE = NCHUNK * CHUNK     # 25600
BCH = 96
ECOLS = NCHUNK * BCH       # 1280

_cache = {}


def _build(steps, with_bias, do_collective=True, do_publish=True, do_chunks=True):
    nc = bacc.Bacc(None)
    f32, f16, i16 = mybir.dt.float32, mybir.dt.float16, mybir.dt.int16

    # compact, split inputs (transfer wall ~ largest single param)
    wm_in = [nc.declare_dram_parameter(f"wm{j}", [16, CHUNK // 2], f16,
                                       isOutput=False) for j in range(2 * NCHUNK)]
    ix_in = [nc.declare_dram_parameter(f"ix{c}", [128, CHUNK // 16], i16,
                                       isOutput=False) for c in range(NCHUNK)]
    v0_in = [nc.declare_dram_parameter(f"v0{j}", [8, ECOLS // 2], f32,
                                       isOutput=False) for j in range(16)]
    fold_in = [nc.declare_dram_parameter(f"fold{j}", [32, 128], f32,
                                         isOutput=False) for j in range(4)]
    mdt_in = [nc.declare_dram_parameter(f"mdt{j}", [8, ECOLS // 2], f32,
                                        isOutput=False) for j in range(2)]
    if with_bias:
        bm_in = [nc.declare_dram_parameter(f"bm{j}", [8, ECOLS // 2], f32,
                                           isOutput=False) for j in range(2)]
    bidx_in = nc.declare_dram_parameter("bidx", [128, ECOLS // 16], i16, isOutput=False)
    idx3_in = nc.declare_dram_parameter("idx3", [128, MCORE // 16], i16, isOutput=False)
    vout = nc.declare_dram_parameter("vout", [B, NDEVT], f16, isOutput=True)

    with TileContext(nc) as tc:
        with (
            tc.tile_pool(name="big", bufs=1) as big,
            tc.tile_pool(name="gbuf", bufs=2) as gbuf,
            tc.tile_pool(name="wbuf", bufs=2) as wbuf,
            tc.tile_pool(name="cs", bufs=2) as csp,
            tc.tile_pool(name="small", bufs=1) as small,
            tc.tile_pool(name="psum", bufs=1, space="PSUM") as psum,
            tc.tile_pool(name="dram", bufs=1, space="DRAM") as dram,
            tc.tile_pool(name="agpool", bufs=max(steps, 1), space="DRAM") as agp,
        ):
            tbl = big.tile([128, NH], f32, tag="tbl")
            idx1 = small.tile([128, LCORE // 16], i16, tag="idx1")
            bidx = small.tile([128, ECOLS // 16], i16, tag="bidx")
            idx3 = small.tile([128, MCORE // 16], i16, tag="idx3")
            v = small.tile([128, ECOLS], f32, tag="v")
            mdt = small.tile([128, ECOLS], f32, tag="mdt")
            fold = small.tile([128, 128], f32, tag="fold")
            bm = small.tile([128, ECOLS], f32, tag="bm") if with_bias else None
            E = small.tile([128, ECOLS], f32, tag="E")
            syn = small.tile([128, ECOLS], f32, tag="syn")
            carry = small.tile([128, 1], f32, tag="carry")
            rslab = small.tile([128, MCORE], f32, tag="rslab")
            vslab = small.tile([128, MCORE], f32, tag="vslab")

            slab_d = dram.tile([B, NDEVT], f32)
            agl_d = dram.tile([NDEV * B, NDEVT], f32)
            wmx = dram.tile([128, LCORE], f16)

            # ---- input loads / on-device expansion (one-time) ----
            for c in range(NCHUNK):
                nc.sync.dma_start(out=idx1[:, c * 100:(c + 1) * 100], in_=ix_in[c][:])
            nc.sync.dma_start(out=bidx[:], in_=bidx_in[:])
            nc.sync.dma_start(out=idx3[:], in_=idx3_in[:])
            for j in range(4):
                nc.sync.dma_start(out=fold[32 * j:32 * (j + 1), :], in_=fold_in[j][:])
            HC = ECOLS // 2
            mdtv = mdt[:].rearrange("(k r) n -> k r n", k=8)
            for r in range(16):
                nc.sync.dma_start(out=mdtv[:, r, 0:HC], in_=mdt_in[0][:])
                nc.sync.dma_start(out=mdtv[:, r, HC:ECOLS], in_=mdt_in[1][:])
            if with_bias:
                bmv = bm[:].rearrange("(k r) n -> k r n", k=8)
                for r in range(16):
                    nc.sync.dma_start(out=bmv[:, r, 0:HC], in_=bm_in[0][:])
                    nc.sync.dma_start(out=bmv[:, r, HC:ECOLS], in_=bm_in[1][:])
            vv = v[:].rearrange("(k h b) n -> k h b n", k=8, h=2)
            for h in range(2):
                for b in range(B):
                    nc.sync.dma_start(out=vv[:, h, b, 0:HC], in_=v0_in[2 * b][:])
                    nc.sync.dma_start(out=vv[:, h, b, HC:ECOLS], in_=v0_in[2 * b + 1][:])
            # expand weights (batch-invariant rows) into DRAM wmx once
            for c in range(NCHUNK):
                w = wbuf.tile([128, CHUNK], f16, tag="w")
                wv = w[:].rearrange("(g b) n -> g b n", g=16)
                for b in range(B):
                    nc.sync.dma_start(out=wv[:, b, 0:CHUNK // 2], in_=wm_in[2 * c][:])
                    nc.sync.dma_start(out=wv[:, b, CHUNK // 2:CHUNK],
                                      in_=wm_in[2 * c + 1][:])
                nc.sync.dma_start(out=wmx[:, c * CHUNK:(c + 1) * CHUNK], in_=w[:])
            nc.vector.memset(carry[:], 0.0)

            def publish():
                # r = relu(v) -> id-order slab -> DRAM -> AllGather -> tables
                nc.scalar.activation(syn[:], v[:], mybir.ActivationFunctionType.Relu)
                nc.gpsimd.ap_gather(out_ap=rslab[:], in_ap=syn[:], idxs_ap=idx3[:],
                                    channels=128, num_elems=ECOLS, d=1, num_idxs=MCORE)
                for k in range(NCORES):
                    nc.sync.dma_start(out=slab_d[:, k * MCORE:(k + 1) * MCORE],
                                      in_=rslab[16 * k:16 * k + 8, :])
                ag_d = agp.tile([NDEV * B, NDEVT], f32, addr_space="Shared", tag="ag")
                if do_collective:
                    nc.gpsimd.collective_compute(
                        "AllGather", mybir.AluOpType.bypass,
                        replica_groups=[list(range(NDEV))],
                        ins=[slab_d[:]], outs=[ag_d[:]],
                    )
                # bounce collective output (Shared memory reads are ~4GB/s)
                # to regular DRAM once, then broadcast-rebuild from there
                nc.sync.dma_start(out=agl_d[:], in_=ag_d[:])
                agv = agl_d[:].rearrange("(d b) n -> d b n", d=NDEV)
                for h in range(2):
                    for b in range(B):
                        nc.sync.dma_start(
                            out=tbl[:].rearrange("(k r) n -> k r n", k=8)[:, 8 * h + b, :],
                            in_=agv[4 * h:4 * h + 4, b, :][None]
                                .to_broadcast([8, 4, NDEVT]),
                        )

            def step_body(last):
                if not do_chunks:
                    if not last and do_publish:
                        publish()
                    return
                s_tiles = {}
                for c in range(NCHUNK):
                    g = gbuf.tile([128, CHUNK], f32, tag="g")
                    w = wbuf.tile([128, CHUNK], f16, tag="w")
                    cur = csp.tile([128, CHUNK], f32, tag="cur")
                    s = csp.tile([128, CHUNK], f32, tag="s")
                    s_tiles[c] = s
                    nc.sync.dma_start(out=w[:], in_=wmx[:, c * CHUNK:(c + 1) * CHUNK])
                    nc.gpsimd.ap_gather(
                        out_ap=g[:], in_ap=tbl[:],
                        idxs_ap=idx1[:, c * (CHUNK // 16):(c + 1) * (CHUNK // 16)],
                        channels=128, num_elems=NH, d=1, num_idxs=CHUNK,
                    )
                    # extract previous chunk's boundaries (after this gather so
                    # POOL doesn't stall on the DVE scan)
                    if c >= 1:
                        sp = s_tiles.pop(c - 1)
                        nc.gpsimd.ap_gather(
                            out_ap=E[:, (c - 1) * BCH:c * BCH], in_ap=sp[:],
                            idxs_ap=bidx[:, (c - 1) * (BCH // 16):c * (BCH // 16)],
                            channels=128, num_elems=CHUNK, d=1, num_idxs=BCH,
                        )
                    nc.vector.tensor_tensor(out=cur[:], in0=g[:], in1=w[:],
                                            op=mybir.AluOpType.mult)
                    init = 0.0 if c == 0 else carry[:, 0:1]
                    nc.vector.tensor_tensor_scan(
                        out=s[:], data0=cur[:], data1=cur[:], initial=init,
                        op0=mybir.AluOpType.add, op1=mybir.AluOpType.bypass,
                    )
                    if c < NCHUNK - 1:
                        nc.vector.tensor_copy(out=carry[:], in_=s[:, CHUNK - 1:CHUNK])
                c = NCHUNK
                sp = s_tiles.pop(c - 1)
                nc.gpsimd.ap_gather(
                    out_ap=E[:, (c - 1) * BCH:c * BCH], in_ap=sp[:],
                    idxs_ap=bidx[:, (c - 1) * (BCH // 16):c * (BCH // 16)],
                    channels=128, num_elems=CHUNK, d=1, num_idxs=BCH,
                )
                # fold halves: Ef = fold.T @ E
                ef = psum.tile([128, ECOLS], f32, tag="ef")
                for j in range(0, ECOLS, 512):
                    jw = min(512, ECOLS - j)
                    nc.tensor.matmul(out=ef[:, j:j + jw], lhsT=fold[:],
                                     rhs=E[:, j:j + jw], start=True, stop=True)
                # segment sums by first difference (via SBUF copy of ef)
                nc.vector.tensor_copy(out=syn[:], in_=ef[:])
                nc.vector.tensor_copy(out=E[:, 0:1], in_=syn[:, 0:1])
                nc.vector.tensor_tensor(out=E[:, 1:ECOLS], in0=syn[:, 1:ECOLS],
                                        in1=syn[:, 0:ECOLS - 1],
                                        op=mybir.AluOpType.subtract)
                # v += mdt * (E - v) (+ bm)
                nc.vector.tensor_tensor(out=E[:], in0=E[:], in1=v[:],
                                        op=mybir.AluOpType.subtract)
                nc.vector.tensor_tensor(out=E[:], in0=E[:], in1=mdt[:],
                                        op=mybir.AluOpType.mult)
                nc.vector.tensor_tensor(out=v[:], in0=v[:], in1=E[:],
                                        op=mybir.AluOpType.add)
                if with_bias:
                    nc.vector.tensor_tensor(out=v[:], in0=v[:], in1=bm[:],
                                            op=mybir.AluOpType.add)
                if not last and do_publish:
                    publish()

            if steps >= 1 and do_publish:
                publish()
                for s in range(steps):
                    step_body(last=(s == steps - 1))

            nc.gpsimd.ap_gather(out_ap=vslab[:], in_ap=v[:], idxs_ap=idx3[:],
                                channels=128, num_elems=ECOLS, d=1, num_idxs=MCORE)
            vslab16 = small.tile([128, MCORE], f16, tag="vslab16")
            nc.vector.tensor_copy(out=vslab16[:], in_=vslab[:])
            for k in range(NCORES):
                nc.sync.dma_start(out=vout[:, k * MCORE:(k + 1) * MCORE],
                                  in_=vslab16[16 * k:16 * k + 8, :])
    nc.finalize()
    return nc


def _wrap16(a):
    out = np.zeros((128, a.shape[1] // 16), a.dtype)
    for k in range(8):
        for p in range(16):
            out[16 * k + p] = a[k, p::16]
    return out


def _prep(tm1_input, v_init, weights, bias, tau_params, scale_excitatory,
          scale_inhibitory, source_indices, target_indices, type_ids,
          tm1_indices, steps):
    one = np.float32(1.0)
    weights = np.asarray(weights, np.float32)
    es = np.where(weights > 0, np.float32(scale_excitatory),
                  np.where(weights < 0, np.float32(scale_inhibitory), one))
    sw = (weights * es).astype(np.float32)

    type_ids = np.asarray(type_ids)
    tau = np.asarray(tau_params, np.float32)[type_ids]
    taup = np.concatenate([tau, np.full(N - NREAL, 1.0, np.float32)])
    is_tm1 = np.zeros(N, bool)
    tm1_indices = np.asarray(tm1_indices)
    is_tm1[tm1_indices] = True
    biasp = np.zeros(N, np.float32)
    biasp[:NREAL] = np.asarray(bias, np.float32)

    vc = np.zeros((B, N), np.float32)
    vc[:, :NREAL] = np.asarray(v_init, np.float32)
    vc[:, tm1_indices] = np.asarray(tm1_input, np.float32)

    order = np.argsort(target_indices, kind="stable")
    tsrc = np.asarray(source_indices)[order].astype(np.int64)
    tw = sw[order]
    ttgt = np.asarray(target_indices)[order].astype(np.int64)
    t_starts = np.searchsorted(ttgt, np.arange(N + 1, dtype=np.int64), side="left")

    F = np.zeros((128, 128), np.float32)
    for p in range(128):
        for m in range(128):
            if p // 16 == m // 16 and p % 8 == m % 8:
                F[p, m] = 1.0

    in_maps = []
    meta = []
    HC = ECOLS // 2
    for d in range(NDEV):
        idx1 = np.zeros((8, LCORE), np.int16)
        wmc = np.zeros((16, LCORE), np.float16)   # row g = 2k + h
        bpos = np.zeros((8, ECOLS), np.int16)
        col_of_t = np.zeros((8, MCORE), np.int64)
        for k in range(NCORES):
            t0 = d * NDEVT + k * MCORE
            e0, e1 = t_starts[t0], t_starts[t0 + MCORE]
            srcs = tsrc[e0:e1]
            ws = tw[e0:e1]
            counts = t_starts[t0 + 1:t0 + MCORE + 1] - t_starts[t0:t0 + MCORE]
            pos = np.cumsum(counts)              # extract position per target
            nslots = 1 + len(srcs)               # sentinel at slot 0
            assert nslots <= LCORE, f"core slots {nslots} > {LCORE}"
            idx1[k, 1:nslots] = (srcs % NH).astype(np.int16)
            half = np.zeros(LCORE, np.int64)
            wrow = np.zeros(LCORE, np.float32)
            half[1:nslots] = srcs // NH
            wrow[1:nslots] = ws
            for h in range(2):
                wmc[2 * k + h] = np.where(half == h, wrow, 0.0).astype(np.float16)
            # boundary extraction, chunked (real targets only; virtual
            # padding targets share the final pad column: syn there is
            # garbage but mdt=0 and v0=0 keep their state at 0)
            ids_k = d * NDEVT + k * MCORE + np.arange(MCORE)
            cchunk = pos // CHUNK
            clocal = pos % CHUNK
            ci = 0
            for c in range(NCHUNK):
                nhere = 0
                while ci < MCORE and cchunk[ci] == c:
                    if ids_k[ci] >= NREAL:
                        col_of_t[k, ci] = ECOLS - 1
                        ci += 1
                        continue
                    assert nhere < BCH - 1, f"chunk {c} boundary overflow"
                    bpos[k, c * BCH + nhere] = clocal[ci]
                    col_of_t[k, ci] = c * BCH + nhere
                    nhere += 1
                    ci += 1
                padv = bpos[k, c * BCH + nhere - 1] if nhere else 0
                bpos[k, c * BCH + nhere:(c + 1) * BCH] = padv
            assert ci == MCORE
        gids = (d * NDEVT + np.arange(NDEVT)).reshape(NCORES, MCORE)
        v0 = np.zeros((8 * B, ECOLS), np.float32)   # row 8k + b
        mdt = np.zeros((8, ECOLS), np.float32)      # row k
        bmt = np.zeros((8, ECOLS), np.float32)
        for k in range(NCORES):
            cols = col_of_t[k]
            ids = gids[k]
            upd = (~is_tm1[ids]) & (ids < NREAL)
            mvals = np.where(upd, DT / taup[ids], 0.0).astype(np.float32)
            mdt[k, cols] = mvals
            bmt[k, cols] = (mvals * biasp[ids]).astype(np.float32)
            for b in range(B):
                v0[8 * k + b, cols] = vc[b, ids]
        m = {
            "bidx": _wrap16(bpos), "idx3": _wrap16(col_of_t.astype(np.int16)),
        }
        ix_w = _wrap16(idx1)
        for c in range(NCHUNK):
            m[f"ix{c}"] = np.ascontiguousarray(ix_w[:, c * 100:(c + 1) * 100])
        for j in range(2 * NCHUNK):
            m[f"wm{j}"] = np.ascontiguousarray(
                wmc[:, j * (CHUNK // 2):(j + 1) * (CHUNK // 2)])
        for b in range(B):
            vb = v0[np.arange(NCORES) * 8 + b]
            m[f"v0{2 * b}"] = np.ascontiguousarray(vb[:, 0:HC])
            m[f"v0{2 * b + 1}"] = np.ascontiguousarray(vb[:, HC:ECOLS])
        for j in range(4):
            m[f"fold{j}"] = np.ascontiguousarray(F[32 * j:32 * (j + 1), :])
        m["mdt0"] = np.ascontiguousarray(mdt[:, 0:HC])
        m["mdt1"] = np.ascontiguousarray(mdt[:, HC:ECOLS])
        m["bm0"] = np.ascontiguousarray(bmt[:, 0:HC])
        m["bm1"] = np.ascontiguousarray(bmt[:, HC:ECOLS])
        in_maps.append(m)
        meta.append(col_of_t)
    return in_maps, meta


def kernel(**inputs):
    steps = int(inputs["steps"])
    bias = np.asarray(inputs["bias"])
    with_bias = bool(np.any(bias != 0))
    in_maps, _meta = _prep(**inputs)
    if not with_bias:
        for m in in_maps:
            m.pop("bm0")
            m.pop("bm1")
    key = (steps, with_bias)
    if key not in _cache:
        _cache[key] = _build(steps, with_bias)
    nc = _cache[key]
    res = run_bass_kernel_spmd(nc, in_maps, list(range(NDEV)))
    out = np.zeros((B, NREAL), np.float32)
    for d in range(NDEV):
        sl = res.results[d]["vout"].astype(np.float32)
        lo = d * NDEVT
        hi = min(lo + NDEVT, NREAL)
        out[:, lo:hi] = sl[:, :hi - lo]
    return out


# revision 23
# speedup vs baseline: 6.4141x; 1.1305x over previous
"""Drosophila optic lobe circuit simulation on 8 Trainium2 NeuronCores.

Edge/target-sharded across 8 devices; batch rides partitions.
- N padded 49000->49152 = 8 dev x 8 cores x 768 targets.
- Gather tables: partition p = 16k + 8h + b holds r=relu(v) of batch b,
  source-half h ([24576] fp32). ap_gather per 1600-slot chunk fetches
  r[src] for all 8 batches; wrong-half rows masked by wmask=0.
- currents = gathered * wmask (fp16 static weights streamed from DRAM).
- Scatter-add = carried inclusive cumsum (tensor_tensor_scan) + boundary
  extraction (small ap_gather per chunk) + first difference.
- Source halves folded by a static 0/1 matmul; v updated in extract-column
  layout; r re-sharded to id order (ap_gather), AllGathered across devices,
  tables rebuilt by broadcast DMAs.

Host->device transfer over the axon tunnel is the wall-clock bottleneck
and scales with the LARGEST single parameter, not total bytes (arrays
stream in parallel). So inputs are shipped deduplicated (weights are
batch-invariant: 16 unique rows, not 128; v0 is half-invariant; mdt is
per-core) and split into ~200KB chunks. The r-table (12.6MB, formerly an
input) is built on device by running the publish path once before the
loop. Weights are expanded once into device DRAM (wmx) and streamed
per-chunk from there each step, keeping the steady-state instruction
stream unchanged.
"""

import numpy as np
import sys

sys.path.insert(0, "/opt/trn_rl_repo")

import jax

import concourse.bacc as bacc
import concourse.mybir as mybir
from concourse.tile import TileContext
from concourse.bass_utils import run_bass_kernel_spmd

# Cache XLA executables on disk so repeat dispatches skip the re-compile
# path (the jit closure inside run_bass_via_pjrt is fresh per call, so
# jax's in-memory caches never hit).
jax.config.update("jax_compilation_cache_dir", "/tmp/jax_cache_bass")
jax.config.update("jax_persistent_cache_min_compile_time_secs", 0.0)
jax.config.update("jax_persistent_cache_min_entry_size_bytes", 0)

NREAL = 49000
B = 8
DT = 0.1
NDEV = 8
N = 49152
NH = N // 2
NDEVT = N // NDEV          # 6144
NCORES = 8
MCORE = NDEVT // NCORES    # 768
NCHUNK = 16
CHUNK = 1600
LCORE = NCHUNK * CHUNK     # 25600
BCH = 96
ECOLS = NCHUNK * BCH       # 1280

_cache = {}


def _build(steps, with_bias, do_collective=True, do_publish=True, do_chunks=True,
           ag_shared=True, ag_bounce=True):
    nc = bacc.Bacc(None)
    f32, f16, i16 = mybir.dt.float32, mybir.dt.float16, mybir.dt.int16

    # compact, split inputs (transfer wall ~ largest single param)
    wm_in = [nc.declare_dram_parameter(f"wm{j}", [16, CHUNK // 2], f16,
                                       isOutput=False) for j in range(2 * NCHUNK)]
    ix_in = [nc.declare_dram_parameter(f"ix{c}", [128, CHUNK // 16], i16,
                                       isOutput=False) for c in range(NCHUNK)]
    v0_in = [nc.declare_dram_parameter(f"v0{j}", [8, ECOLS // 2], f32,
                                       isOutput=False) for j in range(16)]
    fold_in = [nc.declare_dram_parameter(f"fold{j}", [32, 128], f32,
                                         isOutput=False) for j in range(4)]
    mdt_in = [nc.declare_dram_parameter(f"mdt{j}", [8, ECOLS // 2], f32,
                                        isOutput=False) for j in range(2)]
    if with_bias:
        bm_in = [nc.declare_dram_parameter(f"bm{j}", [8, ECOLS // 2], f32,
                                           isOutput=False) for j in range(2)]
    bidx_in = nc.declare_dram_parameter("bidx", [128, ECOLS // 16], i16, isOutput=False)
    idx3_in = nc.declare_dram_parameter("idx3", [128, MCORE // 16], i16, isOutput=False)
    fot_in = nc.declare_dram_parameter("fot", [16, 128], f16, isOutput=False)
    vout = nc.declare_dram_parameter("vout", [B, NDEVT], f16, isOutput=True)

    with TileContext(nc) as tc:
        with (
            tc.tile_pool(name="big", bufs=1) as big,
            tc.tile_pool(name="gbuf", bufs=2) as gbuf,
            tc.tile_pool(name="wbuf", bufs=2) as wbuf,
            tc.tile_pool(name="cs", bufs=2) as csp,
            tc.tile_pool(name="rs", bufs=2) as rsp,
            tc.tile_pool(name="small", bufs=1) as small,
            tc.tile_pool(name="psum", bufs=1, space="PSUM") as psum,
            tc.tile_pool(name="dram", bufs=1, space="DRAM") as dram,
            tc.tile_pool(name="agpool", bufs=max(steps, 1), space="DRAM") as agp,
        ):
            tbl = big.tile([128, NH], f32, tag="tbl")
            fot = small.tile([16, 128], f16, tag="fot")
            rslab16 = small.tile([128, MCORE], f16, tag="rslab16")
            idx1 = small.tile([128, LCORE // 16], i16, tag="idx1")
            bidx = small.tile([128, ECOLS // 16], i16, tag="bidx")
            idx3 = small.tile([128, MCORE // 16], i16, tag="idx3")
            v = small.tile([128, ECOLS], f32, tag="v")
            mdt = small.tile([128, ECOLS], f32, tag="mdt")
            fold = small.tile([128, 128], f32, tag="fold")
            bm = small.tile([128, ECOLS], f32, tag="bm") if with_bias else None
            E = small.tile([128, ECOLS], f32, tag="E")
            syn = small.tile([128, ECOLS], f32, tag="syn")
            carry = small.tile([128, 1], f32, tag="carry")
            rslab = small.tile([128, MCORE], f32, tag="rslab")
            vslab = small.tile([128, MCORE], f32, tag="vslab")

            slab_d = dram.tile([B, NDEVT], f16)
            wmx = dram.tile([128, LCORE], f16)

            # ---- input loads / on-device expansion (one-time) ----
            for c in range(NCHUNK):
                nc.sync.dma_start(out=idx1[:, c * 100:(c + 1) * 100], in_=ix_in[c][:])
            nc.sync.dma_start(out=bidx[:], in_=bidx_in[:])
            nc.sync.dma_start(out=idx3[:], in_=idx3_in[:])
            nc.sync.dma_start(out=fot[:], in_=fot_in[:])
            for j in range(4):
                nc.sync.dma_start(out=fold[32 * j:32 * (j + 1), :], in_=fold_in[j][:])
            HC = ECOLS // 2
            mdtv = mdt[:].rearrange("(k r) n -> k r n", k=8)
            for r in range(16):
                nc.sync.dma_start(out=mdtv[:, r, 0:HC], in_=mdt_in[0][:])
                nc.sync.dma_start(out=mdtv[:, r, HC:ECOLS], in_=mdt_in[1][:])
            if with_bias:
                bmv = bm[:].rearrange("(k r) n -> k r n", k=8)
                for r in range(16):
                    nc.sync.dma_start(out=bmv[:, r, 0:HC], in_=bm_in[0][:])
                    nc.sync.dma_start(out=bmv[:, r, HC:ECOLS], in_=bm_in[1][:])
            vv = v[:].rearrange("(k h b) n -> k h b n", k=8, h=2)
            for h in range(2):
                for b in range(B):
                    nc.sync.dma_start(out=vv[:, h, b, 0:HC], in_=v0_in[2 * b][:])
                    nc.sync.dma_start(out=vv[:, h, b, HC:ECOLS], in_=v0_in[2 * b + 1][:])
            # expand weights (batch-invariant rows) into DRAM wmx once
            for c in range(NCHUNK):
                w = wbuf.tile([128, CHUNK], f16, tag="w")
                wv = w[:].rearrange("(g b) n -> g b n", g=16)
                for b in range(B):
                    nc.sync.dma_start(out=wv[:, b, 0:CHUNK // 2], in_=wm_in[2 * c][:])
                    nc.sync.dma_start(out=wv[:, b, CHUNK // 2:CHUNK],
                                      in_=wm_in[2 * c + 1][:])
                nc.sync.dma_start(out=wmx[:, c * CHUNK:(c + 1) * CHUNK], in_=w[:])
            nc.vector.memset(carry[:], 0.0)

            RC = 2048

            def publish():
                # r = relu(v) -> id-order f16 slab -> DRAM -> AllGather(f16)
                nc.scalar.activation(syn[:], v[:], mybir.ActivationFunctionType.Relu)
                nc.gpsimd.ap_gather(out_ap=rslab[:], in_ap=syn[:], idxs_ap=idx3[:],
                                    channels=128, num_elems=ECOLS, d=1, num_idxs=MCORE)
                nc.vector.tensor_copy(out=rslab16[:], in_=rslab[:])
                for k in range(NCORES):
                    nc.sync.dma_start(out=slab_d[:, k * MCORE:(k + 1) * MCORE],
                                      in_=rslab16[16 * k:16 * k + 8, :])
                if ag_shared:
                    ag_d = agp.tile([NDEV * B, NDEVT], f16, addr_space="Shared", tag="ag")
                else:
                    ag_d = agp.tile([NDEV * B, NDEVT], f16, tag="ag")
                if do_collective:
                    nc.gpsimd.collective_compute(
                        "AllGather", mybir.AluOpType.bypass,
                        replica_groups=[list(range(NDEV))],
                        ins=[slab_d[:]], outs=[ag_d[:]],
                    )
                # table rebuild via PE partition-broadcast: per 2048-col chunk
                # (dd, c2), rsrc16[8h+b, n] = ag[(4h+dd)*8+b, n] (2 plain
                # DMAs), then tbl[:, cols] = fot.T @ rsrc16 replicates the 16
                # rows to all 128 partitions and converts f16->f32 in PSUM.
                for dd in range(4):
                    for c2 in range(NDEVT // RC):
                        rs = rsp.tile([16, RC], f16, tag="rs")
                        pr = psum.tile([128, RC], f32, tag="pr")
                        for h in range(2):
                            eng = nc.sync if h == 0 else nc.scalar
                            eng.dma_start(
                                out=rs[8 * h:8 * h + 8, :],
                                in_=ag_d[(4 * h + dd) * 8:(4 * h + dd) * 8 + 8,
                                         c2 * RC:(c2 + 1) * RC])
                        for j in range(0, RC, 512):
                            nc.tensor.matmul(out=pr[:, j:j + 512], lhsT=fot[:],
                                             rhs=rs[:, j:j + 512],
                                             start=True, stop=True)
                        nc.scalar.activation(
                            tbl[:, dd * NDEVT + c2 * RC:dd * NDEVT + (c2 + 1) * RC],
                            pr[:], mybir.ActivationFunctionType.Copy)

            def step_body(last):
                if not do_chunks:
                    if not last and do_publish:
                        publish()
                    return
                s_tiles = {}
                for c in range(NCHUNK):
                    g = gbuf.tile([128, CHUNK], f32, tag="g")
                    w = wbuf.tile([128, CHUNK], f16, tag="w")
                    cur = csp.tile([128, CHUNK], f32, tag="cur")
                    s = csp.tile([128, CHUNK], f32, tag="s")
                    s_tiles[c] = s
                    nc.sync.dma_start(out=w[:], in_=wmx[:, c * CHUNK:(c + 1) * CHUNK])
                    nc.gpsimd.ap_gather(
                        out_ap=g[:], in_ap=tbl[:],
                        idxs_ap=idx1[:, c * (CHUNK // 16):(c + 1) * (CHUNK // 16)],
                        channels=128, num_elems=NH, d=1, num_idxs=CHUNK,
                    )
                    # extract previous chunk's boundaries (after this gather so
                    # POOL doesn't stall on the DVE scan)
                    if c >= 1:
                        sp = s_tiles.pop(c - 1)
                        nc.gpsimd.ap_gather(
                            out_ap=E[:, (c - 1) * BCH:c * BCH], in_ap=sp[:],
                            idxs_ap=bidx[:, (c - 1) * (BCH // 16):c * (BCH // 16)],
                            channels=128, num_elems=CHUNK, d=1, num_idxs=BCH,
                        )
                    nc.vector.tensor_tensor(out=cur[:], in0=g[:], in1=w[:],
                                            op=mybir.AluOpType.mult)
                    init = 0.0 if c == 0 else carry[:, 0:1]
                    nc.vector.tensor_tensor_scan(
                        out=s[:], data0=cur[:], data1=cur[:], initial=init,
                        op0=mybir.AluOpType.add, op1=mybir.AluOpType.bypass,
                    )
                    if c < NCHUNK - 1:
                        nc.vector.tensor_copy(out=carry[:], in_=s[:, CHUNK - 1:CHUNK])
                c = NCHUNK
                sp = s_tiles.pop(c - 1)
                nc.gpsimd.ap_gather(
                    out_ap=E[:, (c - 1) * BCH:c * BCH], in_ap=sp[:],
                    idxs_ap=bidx[:, (c - 1) * (BCH // 16):c * (BCH // 16)],
                    channels=128, num_elems=CHUNK, d=1, num_idxs=BCH,
                )
                # fold halves: Ef = fold.T @ E
                ef = psum.tile([128, ECOLS], f32, tag="ef")
                for j in range(0, ECOLS, 512):
                    jw = min(512, ECOLS - j)
                    nc.tensor.matmul(out=ef[:, j:j + jw], lhsT=fold[:],
                                     rhs=E[:, j:j + jw], start=True, stop=True)
                # segment sums by first difference (via SBUF copy of ef)
                nc.vector.tensor_copy(out=syn[:], in_=ef[:])
                nc.vector.tensor_copy(out=E[:, 0:1], in_=syn[:, 0:1])
                nc.vector.tensor_tensor(out=E[:, 1:ECOLS], in0=syn[:, 1:ECOLS],
                                        in1=syn[:, 0:ECOLS - 1],
                                        op=mybir.AluOpType.subtract)
                # v += mdt * (E - v) (+ bm)
                nc.vector.tensor_tensor(out=E[:], in0=E[:], in1=v[:],
                                        op=mybir.AluOpType.subtract)
                nc.vector.tensor_tensor(out=E[:], in0=E[:], in1=mdt[:],
                                        op=mybir.AluOpType.mult)
                nc.vector.tensor_tensor(out=v[:], in0=v[:], in1=E[:],
                                        op=mybir.AluOpType.add)
                if with_bias:
                    nc.vector.tensor_tensor(out=v[:], in0=v[:], in1=bm[:],
                                            op=mybir.AluOpType.add)
                if not last and do_publish:
                    publish()

            if steps >= 1 and do_publish:
                publish()
                for s in range(steps):
                    step_body(last=(s == steps - 1))

            nc.gpsimd.ap_gather(out_ap=vslab[:], in_ap=v[:], idxs_ap=idx3[:],
                                channels=128, num_elems=ECOLS, d=1, num_idxs=MCORE)
            vslab16 = small.tile([128, MCORE], f16, tag="vslab16")
            nc.vector.tensor_copy(out=vslab16[:], in_=vslab[:])
            for k in range(NCORES):
                nc.sync.dma_start(out=vout[:, k * MCORE:(k + 1) * MCORE],
                                  in_=vslab16[16 * k:16 * k + 8, :])
    nc.finalize()
    return nc


def _wrap16(a):
    out = np.zeros((128, a.shape[1] // 16), a.dtype)
    for k in range(8):
        for p in range(16):
            out[16 * k + p] = a[k, p::16]
    return out


def _prep(tm1_input, v_init, weights, bias, tau_params, scale_excitatory,
          scale_inhibitory, source_indices, target_indices, type_ids,
          tm1_indices, steps):
    one = np.float32(1.0)
    weights = np.asarray(weights, np.float32)
    es = np.where(weights > 0, np.float32(scale_excitatory),
                  np.where(weights < 0, np.float32(scale_inhibitory), one))
    sw = (weights * es).astype(np.float32)

    type_ids = np.asarray(type_ids)
    tau = np.asarray(tau_params, np.float32)[type_ids]
    taup = np.concatenate([tau, np.full(N - NREAL, 1.0, np.float32)])
    is_tm1 = np.zeros(N, bool)
    tm1_indices = np.asarray(tm1_indices)
    is_tm1[tm1_indices] = True
    biasp = np.zeros(N, np.float32)
    biasp[:NREAL] = np.asarray(bias, np.float32)

    vc = np.zeros((B, N), np.float32)
    vc[:, :NREAL] = np.asarray(v_init, np.float32)
    vc[:, tm1_indices] = np.asarray(tm1_input, np.float32)

    order = np.argsort(target_indices, kind="stable")
    tsrc = np.asarray(source_indices)[order].astype(np.int64)
    tw = sw[order]
    ttgt = np.asarray(target_indices)[order].astype(np.int64)
    t_starts = np.searchsorted(ttgt, np.arange(N + 1, dtype=np.int64), side="left")

    F = np.zeros((128, 128), np.float32)
    for p in range(128):
        for m in range(128):
            if p // 16 == m // 16 and p % 8 == m % 8:
                F[p, m] = 1.0

    in_maps = []
    meta = []
    HC = ECOLS // 2
    for d in range(NDEV):
        idx1 = np.zeros((8, LCORE), np.int16)
        wmc = np.zeros((16, LCORE), np.float16)   # row g = 2k + h
        bpos = np.zeros((8, ECOLS), np.int16)
        col_of_t = np.zeros((8, MCORE), np.int64)
        for k in range(NCORES):
            t0 = d * NDEVT + k * MCORE
            e0, e1 = t_starts[t0], t_starts[t0 + MCORE]
            srcs = tsrc[e0:e1]
            ws = tw[e0:e1]
            counts = t_starts[t0 + 1:t0 + MCORE + 1] - t_starts[t0:t0 + MCORE]
            pos = np.cumsum(counts)              # extract position per target
            nslots = 1 + len(srcs)               # sentinel at slot 0
            assert nslots <= LCORE, f"core slots {nslots} > {LCORE}"
            idx1[k, 1:nslots] = (srcs % NH).astype(np.int16)
            half = np.zeros(LCORE, np.int64)
            wrow = np.zeros(LCORE, np.float32)
            half[1:nslots] = srcs // NH
            wrow[1:nslots] = ws
            for h in range(2):
                wmc[2 * k + h] = np.where(half == h, wrow, 0.0).astype(np.float16)
            # boundary extraction, chunked (real targets only; virtual
            # padding targets share the final pad column: syn there is
            # garbage but mdt=0 and v0=0 keep their state at 0)
            ids_k = d * NDEVT + k * MCORE + np.arange(MCORE)
            cchunk = pos // CHUNK
            clocal = pos % CHUNK
            ci = 0
            for c in range(NCHUNK):
                nhere = 0
                while ci < MCORE and cchunk[ci] == c:
                    if ids_k[ci] >= NREAL:
                        col_of_t[k, ci] = ECOLS - 1
                        ci += 1
                        continue
                    assert nhere < BCH - 1, f"chunk {c} boundary overflow"
                    bpos[k, c * BCH + nhere] = clocal[ci]
                    col_of_t[k, ci] = c * BCH + nhere
                    nhere += 1
                    ci += 1
                padv = bpos[k, c * BCH + nhere - 1] if nhere else 0
                bpos[k, c * BCH + nhere:(c + 1) * BCH] = padv
            assert ci == MCORE
        gids = (d * NDEVT + np.arange(NDEVT)).reshape(NCORES, MCORE)
        v0 = np.zeros((8 * B, ECOLS), np.float32)   # row 8k + b
        mdt = np.zeros((8, ECOLS), np.float32)      # row k
        bmt = np.zeros((8, ECOLS), np.float32)
        for k in range(NCORES):
            cols = col_of_t[k]
            ids = gids[k]
            upd = (~is_tm1[ids]) & (ids < NREAL)
            mvals = np.where(upd, DT / taup[ids], 0.0).astype(np.float32)
            mdt[k, cols] = mvals
            bmt[k, cols] = (mvals * biasp[ids]).astype(np.float32)
            for b in range(B):
                v0[8 * k + b, cols] = vc[b, ids]
        m = {
            "bidx": _wrap16(bpos), "idx3": _wrap16(col_of_t.astype(np.int16)),
        }
        ix_w = _wrap16(idx1)
        for c in range(NCHUNK):
            m[f"ix{c}"] = np.ascontiguousarray(ix_w[:, c * 100:(c + 1) * 100])
        for j in range(2 * NCHUNK):
            m[f"wm{j}"] = np.ascontiguousarray(
                wmc[:, j * (CHUNK // 2):(j + 1) * (CHUNK // 2)])
        for b in range(B):
            vb = v0[np.arange(NCORES) * 8 + b]
            m[f"v0{2 * b}"] = np.ascontiguousarray(vb[:, 0:HC])
            m[f"v0{2 * b + 1}"] = np.ascontiguousarray(vb[:, HC:ECOLS])
        for j in range(4):
            m[f"fold{j}"] = np.ascontiguousarray(F[32 * j:32 * (j + 1), :])
        m["mdt0"] = np.ascontiguousarray(mdt[:, 0:HC])
        m["mdt1"] = np.ascontiguousarray(mdt[:, HC:ECOLS])
        F2b = np.zeros((16, 128), np.float16)
        F2b[np.arange(128) % 16, np.arange(128)] = 1.0
        m["fot"] = F2b
        m["bm0"] = np.ascontiguousarray(bmt[:, 0:HC])
        m["bm1"] = np.ascontiguousarray(bmt[:, HC:ECOLS])
        in_maps.append(m)
        meta.append(col_of_t)
    return in_maps, meta


def kernel(**inputs):
    steps = int(inputs["steps"])
    bias = np.asarray(inputs["bias"])
    with_bias = bool(np.any(bias != 0))
    in_maps, _meta = _prep(**inputs)
    if not with_bias:
        for m in in_maps:
            m.pop("bm0")
            m.pop("bm1")
    key = (steps, with_bias)
    if key not in _cache:
        _cache[key] = _build(steps, with_bias)
    nc = _cache[key]
    res = run_bass_kernel_spmd(nc, in_maps, list(range(NDEV)))
    out = np.zeros((B, NREAL), np.float32)
    for d in range(NDEV):
        sl = res.results[d]["vout"].astype(np.float32)
        lo = d * NDEVT
        hi = min(lo + NDEVT, NREAL)
        out[:, lo:hi] = sl[:, :hi - lo]
    return out
